# revision 20
# baseline (speedup 1.0000x reference)
"""Multi-head attention (B=2, S=2048, D=1024, H=16, HD=64) on 8 TRN2 NeuronCores.

Sharding: data-parallel over batch (2) x tensor-parallel over head groups (4).
Core c handles batch b = c // 4, local heads hg*4 .. hg*4+3 where hg = c % 4.

Per-core device flow (all matmuls bf16, accumulation fp32 in PSUM):
  Phase Q: qT/kT = (x W_q/k^T)^T via PE (contraction over D), v = x W_v^T.
           q/k stored transposed ([hd, s], head pairs packed at partitions
           0-63 / 64-127); v stored [s, hd] with a ones column appended.
  Phase A: per (head, sq-chunk of 512):
           logitsT[sk, sq] = k^T q on PE (K=hd=64),
           exp via ACT (scale=1/64 folded, PSUM->SBUF, bf16),
           valsT[hd+1, sq] = [v|1]^T exp accumulated over sk tiles
             (row 64 = softmax denominator),
           recip via DVE, broadcast via GPSIMD,
           attnT = exp * recip (DVE) -> DMA to HBM,
           vals row-normalize fused into the PSUM->SBUF copy.
  Phase O: outT[d, s] = W_o^T vals^T on PE (K=64 per head block) -> HBM.

Host: shards/prepacks weights per core, runs SPMD on 8 cores, transposes
attnT/outT shards back and sums the 4 partial outT per batch.
"""
import os
import sys
import types

import numpy as np
import ml_dtypes

import concourse.bacc as bacc
import concourse.mybir as mybir
import concourse.tile as tile
from concourse.bass_utils import run_bass_kernel_spmd

F32 = mybir.dt.float32
BF16 = mybir.dt.bfloat16
EXP = mybir.ActivationFunctionType.Exp

B, S, D = 2, 2048, 1024
H, HD = 16, 64
HL = 4          # heads per core
CH = 512        # sq chunk
P = 128

# Results of the last SPMD run (exec_time_ns etc.), for the test harness.
LAST_RESULTS = None
_NC_CACHE = {}


def _install_ntff_hook():
    """The agent image's antenv lacks axon_hooks; inject it so trace=True
    (BASS_TRACE=1) can capture NTFF profiles under axon."""
    try:
        import antenv
        if "antenv.axon_hooks" in sys.modules:
            return
        mod = types.ModuleType("antenv.axon_hooks")
        mod._hook = None

        def set_axon_ntff_profile_hook(h):
            mod._hook = h

        def get_axon_ntff_profile_hook():
            return mod._hook

        mod.set_axon_ntff_profile_hook = set_axon_ntff_profile_hook
        mod.get_axon_ntff_profile_hook = get_axon_ntff_profile_hook
        sys.modules["antenv.axon_hooks"] = mod
        antenv.axon_hooks = mod
        from trn_agent_boot.trn_boot import _ntff_profile_via_ctypes
        hook = _ntff_profile_via_ctypes('/opt/axon/libaxon_pjrt.so')
        if hook is not None:
            set_axon_ntff_profile_hook(hook)
    except Exception:
        pass


def build_nc(s=S, d=D, hl=HL, hd=HD, ch=CH):
    """Build the per-core Bass program. Parameterized so a small config can be
    checked in CoreSim; the full size is (2048, 1024, 4, 64, 512)."""
    nkt = d // P                  # k-tiles over model dim
    nst = s // P                  # s tiles (also sk tiles)
    nch = s // ch                 # sq chunks
    hp = hl // 2                  # head pairs
    vw = hd + 1                   # v block width incl. ones column
    blk = 2                       # sk tiles per logits/exp block

    nc = bacc.Bacc(None, target_bir_lowering=False)

    xT = nc.dram_tensor("xT", [d, s], BF16, kind="ExternalInput")
    # wqk M-tiles of 128 cols: [q_h0|q_h1], [k_h0|k_h1], [q_h2|q_h3], [k_h2|k_h3]
    wqk = nc.dram_tensor("wqk", [d, hl * 2 * hd], BF16, kind="ExternalInput")
    wv = nc.dram_tensor("wv", [d, hl * hd], BF16, kind="ExternalInput")
    wo = nc.dram_tensor("wo", [hl * hd, d], BF16, kind="ExternalInput")
    bqk = nc.dram_tensor("bqk", [P, hl], F32, kind="ExternalInput")
    bv = nc.dram_tensor("bv", [1, hl * vw], F32, kind="ExternalInput")
    attnT = nc.dram_tensor("attnT", [hl, s, s], F32, kind="ExternalOutput")
    outT = nc.dram_tensor("outT", [d, s], F32, kind="ExternalOutput")

    with tile.TileContext(nc) as tc:
        with (
            tc.tile_pool(name="sb", bufs=1) as sb,
            tc.tile_pool(name="sb2", bufs=2) as sb2,
            tc.tile_pool(name="ps", bufs=2, space="PSUM") as ps,
        ):
            # ---- loads ----
            # xT lives in two 16KB tiles sharing the "exp" tag/slots with
            # phase A's exp half-tiles (4 slots of 16KB)
            hkt = nkt // 2
            xT_a = sb2.tile([P, hkt, s], BF16, tag="exp", bufs=4)
            nc.sync.dma_start(
                xT_a, xT[0:hkt * P, :].rearrange("(kt p) s -> p kt s", p=P))
            xT_b = sb2.tile([P, nkt - hkt, s], BF16, tag="exp", bufs=4)
            nc.sync.dma_start(
                xT_b, xT[hkt * P:, :].rearrange("(kt p) s -> p kt s", p=P))
            xT_parts = (xT_a, xT_b)

            def xT_t(kt):
                return xT_parts[kt // hkt][:, kt % hkt, :]
            wqk_sb = sb.tile([P, nkt, hl * 2 * hd], BF16)
            nc.sync.dma_start(wqk_sb, wqk[:, :].rearrange("(kt p) e -> p kt e", p=P))
            wv_sb = sb.tile([P, nkt, hl * hd], BF16)
            nc.sync.dma_start(wv_sb, wv[:, :].rearrange("(kt p) e -> p kt e", p=P))
            wo_sb = sb.tile([hd, hl, d], BF16)
            nc.sync.dma_start(wo_sb, wo[:, :].rearrange("(kt p) e -> p kt e", p=hd))
            bqk_sb = sb.tile([P, hl], F32)
            nc.sync.dma_start(bqk_sb, bqk[:, :])
            bv_sb = sb.tile([1, hl * vw], F32)
            nc.sync.dma_start(bv_sb, bv[:, :])

            # ---- phase Q: projections ----
            q_sb = sb.tile([P, hp, s], BF16)      # head pair p: h=2p at part 0-63, h=2p+1 at 64-127
            k_sb = sb.tile([P, hp, s], BF16)
            for mt in range(2 * hp):              # [qp0, kp0, qp1, kp1]
                dst = q_sb if mt % 2 == 0 else k_sb
                for c in range(nch):
                    pq = ps.tile([P, ch], F32, tag="acc")
                    for kt in range(nkt):
                        nc.tensor.matmul(
                            pq,
                            wqk_sb[:, kt, mt * P:(mt + 1) * P],
                            xT_t(kt)[:, c * ch:(c + 1) * ch],
                            start=(kt == 0), stop=(kt == nkt - 1),
                        )
                    # copy + per-partition bias add (bias is zero in practice)
                    nc.vector.tensor_scalar_add(
                        dst[:, mt // 2, c * ch:(c + 1) * ch], pq,
                        bqk_sb[:, mt:mt + 1])
            v_sb = sb.tile([P, nst, hl * vw], BF16)
            for st in range(nst):
                pv = ps.tile([P, ch], F32, tag="acc")
                for kt in range(nkt):
                    nc.tensor.matmul(
                        pv[:, 0:hl * hd],
                        xT_t(kt)[:, st * P:(st + 1) * P],
                        wv_sb[:, kt, :],
                        start=(kt == 0), stop=(kt == nkt - 1),
                    )
                nc.vector.tensor_copy(
                    v_sb[:, st, :].rearrange("p (h e) -> p h e", e=vw)[:, :, 0:hd],
                    pv[:, 0:hl * hd].rearrange("p (h e) -> p h e", e=hd),
                )
            nc.vector.memset(
                v_sb[:, :, :].rearrange("p st (h e) -> p st h e", e=vw)[:, :, :, hd], 1.0)
            # v bias add (zero in practice): bv broadcast over partitions and st
            bv_bc = sb.tile([P, hl * vw], F32)
            nc.gpsimd.partition_broadcast(bv_bc, bv_sb)
            nc.vector.tensor_add(
                v_sb, v_sb,
                bv_bc[:, :].rearrange("p (o e) -> p o e", o=1)
                .broadcast_to([P, nst, hl * vw]))

            # ---- phase A: attention, one head PAIR at a time ----
            # Heads 2p (partitions 0-63) and 2p+1 (64-127) issue adjacent
            # logits matmuls into disjoint PE row groups -> they execute
            # concurrently. One ACT exp op covers both heads per sk tile.
            # exp lives in two sk-half tiles (bufs=4) so iteration i+2's exp
            # stream only waits on the oldest half's normalize, not a whole
            # iteration.
            vals_sb = sb.tile([hd, hl, s], BF16)
            hst = nst // 2
            cl = ch // P
            for c in range(nch):
                for pr in range(hp):
                    eh = [None, None]
                    pv0 = ps.tile([vw, ch], F32, tag="acc")
                    pv1 = ps.tile([vw, ch], F32, tag="acc2")
                    pv = (pv0, pv1)

                    def vals_mm(skt_):
                        cur_ = eh[skt_ // hst]
                        for hh in range(2):
                            h_ = 2 * pr + hh
                            nc.tensor.matmul(
                                pv[hh],
                                v_sb[:, skt_, h_ * vw:(h_ + 1) * vw],
                                cur_[:, skt_ % hst, hh, :],
                                start=(skt_ == 0), stop=(skt_ == nst - 1),
                            )

                    # software pipeline: vals matmuls run one sk tile behind
                    # the logits/exp, so the in-order PE never waits on ACT.
                    for skt in range(nst):
                        if skt % hst == 0:
                            eh[skt // hst] = sb2.tile([P, hst, 2, ch], BF16,
                                                      tag="exp", bufs=4,
                                                      name="eh")
                        cur = eh[skt // hst]
                        pl = ps.tile([P, 2, ch], F32, tag="l")
                        for hh in range(2):
                            b0 = hh * hd
                            nc.tensor.matmul(
                                pl[:, hh, :],
                                k_sb[b0:b0 + hd, pr, skt * P:(skt + 1) * P],
                                q_sb[b0:b0 + hd, pr, c * ch:(c + 1) * ch],
                                start=True, stop=True,
                            )
                        nc.scalar.activation(cur[:, skt % hst, :, :], pl, EXP,
                                             scale=1.0 / hd)
                        if skt >= 1:
                            vals_mm(skt - 1)
                    vals_mm(nst - 1)

                    # Reciprocal chains for both heads, stages interleaved so
                    # the DMA/gpsimd hops of one head hide under DVE work of
                    # the other. (HW partition_broadcast reads physical
                    # partition 0, and the [1, ch] row is spread over 128
                    # partitions for the reciprocal so all DVE lanes work.)
                    sums64 = [None, None]
                    sums_sq = [None, None]
                    recip_sq = [None, None]
                    recip = [None, None]
                    recip_bc = [None, None]
                    for hh in range(2):
                        sums64[hh] = sb2.tile([P, ch], F32, tag="sums64",
                                              bufs=4, name=f"sums64_{hh}")
                        nc.scalar.copy(sums64[hh][hd:hd + 1, :],
                                       pv[hh][hd:hd + 1, :])
                        sums_sq[hh] = sb2.tile([P, cl], F32, tag="sums_sq",
                                               bufs=4, name=f"sums_sq_{hh}")
                        nc.sync.dma_start(sums_sq[hh], sums64[hh][hd:hd + 1, :])
                    for hh in range(2):
                        recip_sq[hh] = sb2.tile([P, cl], F32, tag="recip_sq",
                                                bufs=4, name=f"recip_sq_{hh}")
                        nc.vector.reciprocal(recip_sq[hh], sums_sq[hh])
                        recip[hh] = sb2.tile([1, ch], F32, tag="recip",
                                             bufs=4, name=f"recip_{hh}")
                        nc.sync.dma_start(recip[hh], recip_sq[hh])
                    for hh in range(2):
                        recip_bc[hh] = sb2.tile([P, ch], F32, tag="rbc",
                                                bufs=4, name=f"recip_bc_{hh}")
                        nc.gpsimd.partition_broadcast(recip_bc[hh], recip[hh])
                    for hh in range(2):
                        h = 2 * pr + hh
                        # vals row-normalize fused into the PSUM->SBUF copy
                        nc.vector.tensor_tensor(
                            vals_sb[:, h, c * ch:(c + 1) * ch],
                            pv[hh][0:hd, :], recip_bc[hh][0:hd, :],
                            op=mybir.AluOpType.mult)
                    # attn normalize + store; half-major so the older exp half
                    # frees its slot first
                    for half in range(2):
                        for hh in range(2):
                            h = 2 * pr + hh
                            attn_st = sb2.tile([P, hst, ch], F32, tag="attn",
                                               bufs=3)
                            nc.vector.tensor_tensor(
                                attn_st,
                                eh[half][:, :, hh, :],
                                recip_bc[hh][:, :]
                                .rearrange("p (o n) -> p o n", o=1)
                                .broadcast_to([P, hst, ch]),
                                op=mybir.AluOpType.mult)
                            nc.sync.dma_start(
                                attnT[h].rearrange("(t p) n -> p t n", p=P)
                                [:, half * hst:(half + 1) * hst,
                                 c * ch:(c + 1) * ch],
                                attn_st)

                # ---- fused output projection for this chunk ----
                o_sb = sb2.tile([P, d // P, ch], F32, tag="attn", bufs=3)
                for mt in range(d // P):
                    po = ps.tile([P, ch], F32, tag="acc")
                    for kt in range(hl):
                        nc.tensor.matmul(
                            po,
                            wo_sb[:, kt, mt * P:(mt + 1) * P],
                            vals_sb[:, kt, c * ch:(c + 1) * ch],
                            start=(kt == 0), stop=(kt == hl - 1),
                        )
                    nc.scalar.copy(o_sb[:, mt, :], po)
                nc.sync.dma_start(
                    outT[:, :].rearrange("(mt p) x -> p mt x", p=P)
                    [:, :, c * ch:(c + 1) * ch], o_sb)


    nc.compile()
    return nc


def _get_nc():
    if "full" not in _NC_CACHE:
        _NC_CACHE["full"] = build_nc()
    return _NC_CACHE["full"]


def kernel(x, w_qkv, b_qkv, w_out, b_out):
    global LAST_RESULTS
    _install_ntff_hook()
    x = np.asarray(x, dtype=np.float32)
    w_qkv = np.asarray(w_qkv, dtype=np.float32)
    b_qkv = np.asarray(b_qkv, dtype=np.float32)
    w_out = np.asarray(w_out, dtype=np.float32)
    b_out = np.asarray(b_out, dtype=np.float32)

    # w_qkv rows are per-head interleaved: row h*192+j -> j<64: q, <128: k, <192: v
    wq = np.stack([w_qkv[g * 3 * HD + 0 * HD: g * 3 * HD + 1 * HD] for g in range(H)])   # [H, 64, D]
    wk = np.stack([w_qkv[g * 3 * HD + 1 * HD: g * 3 * HD + 2 * HD] for g in range(H)])
    wv_ = np.stack([w_qkv[g * 3 * HD + 2 * HD: g * 3 * HD + 3 * HD] for g in range(H)])
    bq = np.stack([b_qkv[g * 3 * HD + 0 * HD: g * 3 * HD + 1 * HD] for g in range(H)])   # [H, 64]
    bk = np.stack([b_qkv[g * 3 * HD + 1 * HD: g * 3 * HD + 2 * HD] for g in range(H)])
    bv_ = np.stack([b_qkv[g * 3 * HD + 2 * HD: g * 3 * HD + 3 * HD] for g in range(H)])

    in_maps = []
    for core in range(8):
        b = core // 4
        hg = core % 4
        gh = [hg * HL + i for i in range(HL)]     # global head ids
        xT = np.ascontiguousarray(x[b].T).astype(ml_dtypes.bfloat16)
        # wqk cols: [q_h0|q_h1], [k_h0|k_h1], [q_h2|q_h3], [k_h2|k_h3]
        cols = []
        bqk_cols = []
        for p_ in range(HL // 2):
            h0, h1 = gh[2 * p_], gh[2 * p_ + 1]
            cols.append(np.concatenate([wq[h0], wq[h1]], axis=0))   # [128, D]
            bqk_cols.append(np.concatenate([bq[h0], bq[h1]]))
            cols.append(np.concatenate([wk[h0], wk[h1]], axis=0))
            bqk_cols.append(np.concatenate([bk[h0], bk[h1]]))
        wqk_arr = np.concatenate(cols, axis=0).T                     # [D, 512]
        bqk_arr = np.stack(bqk_cols, axis=1)                         # [128, 4] (mt order)
        wv_arr = np.concatenate([wv_[g] for g in gh], axis=0).T      # [D, 256]
        bv_arr = np.zeros((1, HL * (HD + 1)), np.float32)
        for i, g in enumerate(gh):
            bv_arr[0, i * (HD + 1): i * (HD + 1) + HD] = bv_[g]
        wo_arr = np.ascontiguousarray(
            w_out[:, gh[0] * HD:(gh[-1] + 1) * HD].T)                # [256, D]
        in_maps.append({
            "xT": xT,
            "wqk": np.ascontiguousarray(wqk_arr).astype(ml_dtypes.bfloat16),
            "wv": np.ascontiguousarray(wv_arr).astype(ml_dtypes.bfloat16),
            "wo": wo_arr.astype(ml_dtypes.bfloat16),
            "bqk": np.ascontiguousarray(bqk_arr),
            "bv": bv_arr,
        })

    nc = _get_nc()
    res = run_bass_kernel_spmd(nc, in_maps, core_ids=list(range(8)))
    LAST_RESULTS = res

    attn = np.empty((B, H, S, S), np.float32)
    out = np.zeros((B, S, D), np.float32)
    for core in range(8):
        b = core // 4
        hg = core % 4
        r = res.results[core]
        for i in range(HL):
            attn[b, hg * HL + i] = r["attnT"][i].T
        out[b] += r["outT"].T
    out += b_out
    return out, attn


# revision 22
# speedup vs baseline: 1.0112x; 1.0112x over previous
"""Multi-head attention (B=2, S=2048, D=1024, H=16, HD=64) on 8 TRN2 NeuronCores.

Sharding: data-parallel over batch (2) x tensor-parallel over head groups (4).
Core c handles batch b = c // 4, local heads hg*4 .. hg*4+3 where hg = c % 4.

Per-core device flow (all matmuls bf16, accumulation fp32 in PSUM):
  Phase Q: qT/kT = (x W_q/k^T)^T via PE (contraction over D), v = x W_v^T.
           q/k stored transposed ([hd, s], head pairs packed at partitions
           0-63 / 64-127); v stored [s, hd] with a ones column appended.
  Phase A: per (head, sq-chunk of 512):
           logitsT[sk, sq] = k^T q on PE (K=hd=64),
           exp via ACT (scale=1/64 folded, PSUM->SBUF, bf16),
           valsT[hd+1, sq] = [v|1]^T exp accumulated over sk tiles
             (row 64 = softmax denominator),
           recip via DVE, broadcast via GPSIMD,
           attnT = exp * recip (DVE) -> DMA to HBM,
           vals row-normalize fused into the PSUM->SBUF copy.
  Phase O: outT[d, s] = W_o^T vals^T on PE (K=64 per head block) -> HBM.

Host: shards/prepacks weights per core, runs SPMD on 8 cores, transposes
attnT/outT shards back and sums the 4 partial outT per batch.
"""
import os
import sys
import types

import numpy as np
import ml_dtypes

import concourse.bacc as bacc
import concourse.mybir as mybir
import concourse.tile as tile
from concourse.bass_utils import run_bass_kernel_spmd

F32 = mybir.dt.float32
BF16 = mybir.dt.bfloat16
EXP = mybir.ActivationFunctionType.Exp

B, S, D = 2, 2048, 1024
H, HD = 16, 64
HL = 4          # heads per core
CH = 512        # sq chunk
P = 128

# Results of the last SPMD run (exec_time_ns etc.), for the test harness.
LAST_RESULTS = None
_NC_CACHE = {}


def _install_ntff_hook():
    """The agent image's antenv lacks axon_hooks; inject it so trace=True
    (BASS_TRACE=1) can capture NTFF profiles under axon."""
    try:
        import antenv
        if "antenv.axon_hooks" in sys.modules:
            return
        mod = types.ModuleType("antenv.axon_hooks")
        mod._hook = None

        def set_axon_ntff_profile_hook(h):
            mod._hook = h

        def get_axon_ntff_profile_hook():
            return mod._hook

        mod.set_axon_ntff_profile_hook = set_axon_ntff_profile_hook
        mod.get_axon_ntff_profile_hook = get_axon_ntff_profile_hook
        sys.modules["antenv.axon_hooks"] = mod
        antenv.axon_hooks = mod
        from trn_agent_boot.trn_boot import _ntff_profile_via_ctypes
        hook = _ntff_profile_via_ctypes('/opt/axon/libaxon_pjrt.so')
        if hook is not None:
            set_axon_ntff_profile_hook(hook)
    except Exception:
        pass


def build_nc(s=S, d=D, hl=HL, hd=HD, ch=CH):
    """Build the per-core Bass program. Parameterized so a small config can be
    checked in CoreSim; the full size is (2048, 1024, 4, 64, 512)."""
    nkt = d // P                  # k-tiles over model dim
    nst = s // P                  # s tiles (also sk tiles)
    nch = s // ch                 # sq chunks
    hp = hl // 2                  # head pairs
    vw = hd + 1                   # v block width incl. ones column
    blk = 2                       # sk tiles per logits/exp block

    nc = bacc.Bacc(None, target_bir_lowering=False)

    xT = nc.dram_tensor("xT", [d, s], BF16, kind="ExternalInput")
    # wqk M-tiles of 128 cols: [q_h0|q_h1], [k_h0|k_h1], [q_h2|q_h3], [k_h2|k_h3]
    wqk = nc.dram_tensor("wqk", [d, hl * 2 * hd], BF16, kind="ExternalInput")
    wv = nc.dram_tensor("wv", [d, hl * hd], BF16, kind="ExternalInput")
    wo = nc.dram_tensor("wo", [hl * hd, d], BF16, kind="ExternalInput")
    bqk = nc.dram_tensor("bqk", [P, hl], F32, kind="ExternalInput")
    bv = nc.dram_tensor("bv", [1, hl * vw], F32, kind="ExternalInput")
    attnT = nc.dram_tensor("attnT", [hl, s, s], F32, kind="ExternalOutput")
    outT = nc.dram_tensor("outT", [d, s], F32, kind="ExternalOutput")

    with tile.TileContext(nc) as tc:
        with (
            tc.tile_pool(name="sb", bufs=1) as sb,
            tc.tile_pool(name="sb2", bufs=2) as sb2,
            tc.tile_pool(name="ps", bufs=2, space="PSUM") as ps,
        ):
            # ---- loads ----
            # xT lives in two 16KB tiles sharing the "exp" tag/slots with
            # phase A's exp half-tiles (4 slots of 16KB)
            hkt = nkt // 2
            xT_a = sb2.tile([P, hkt, s], BF16, tag="exp", bufs=4)
            nc.sync.dma_start(
                xT_a, xT[0:hkt * P, :].rearrange("(kt p) s -> p kt s", p=P))
            xT_b = sb2.tile([P, nkt - hkt, s], BF16, tag="exp", bufs=4)
            nc.sync.dma_start(
                xT_b, xT[hkt * P:, :].rearrange("(kt p) s -> p kt s", p=P))
            xT_parts = (xT_a, xT_b)

            def xT_t(kt):
                return xT_parts[kt // hkt][:, kt % hkt, :]
            wqk_sb = sb.tile([P, nkt, hl * 2 * hd], BF16)
            nc.sync.dma_start(wqk_sb, wqk[:, :].rearrange("(kt p) e -> p kt e", p=P))
            wv_sb = sb.tile([P, nkt, hl * hd], BF16)
            nc.sync.dma_start(wv_sb, wv[:, :].rearrange("(kt p) e -> p kt e", p=P))
            wo_sb = sb.tile([hd, hl, d], BF16)
            nc.sync.dma_start(wo_sb, wo[:, :].rearrange("(kt p) e -> p kt e", p=hd))
            bqk_sb = sb.tile([P, hl], F32)
            nc.sync.dma_start(bqk_sb, bqk[:, :])
            bv_sb = sb.tile([1, hl * vw], F32)
            nc.sync.dma_start(bv_sb, bv[:, :])

            # ---- phase Q: projections ----
            q_sb = sb.tile([P, hp, s], BF16)      # head pair p: h=2p at part 0-63, h=2p+1 at 64-127
            k_sb = sb.tile([P, hp, s], BF16)
            for mt in range(2 * hp):              # [qp0, kp0, qp1, kp1]
                dst = q_sb if mt % 2 == 0 else k_sb
                for c in range(nch):
                    pq = ps.tile([P, ch], F32, tag="acc")
                    for kt in range(nkt):
                        nc.tensor.matmul(
                            pq,
                            wqk_sb[:, kt, mt * P:(mt + 1) * P],
                            xT_t(kt)[:, c * ch:(c + 1) * ch],
                            start=(kt == 0), stop=(kt == nkt - 1),
                        )
                    # copy + per-partition bias add (bias is zero in practice)
                    nc.vector.tensor_scalar_add(
                        dst[:, mt // 2, c * ch:(c + 1) * ch], pq,
                        bqk_sb[:, mt:mt + 1])
            v_sb = sb.tile([P, nst, hl * vw], BF16)
            for st in range(nst):
                pv = ps.tile([P, ch], F32, tag="acc")
                for kt in range(nkt):
                    nc.tensor.matmul(
                        pv[:, 0:hl * hd],
                        xT_t(kt)[:, st * P:(st + 1) * P],
                        wv_sb[:, kt, :],
                        start=(kt == 0), stop=(kt == nkt - 1),
                    )
                nc.vector.tensor_copy(
                    v_sb[:, st, :].rearrange("p (h e) -> p h e", e=vw)[:, :, 0:hd],
                    pv[:, 0:hl * hd].rearrange("p (h e) -> p h e", e=hd),
                )
            nc.vector.memset(
                v_sb[:, :, :].rearrange("p st (h e) -> p st h e", e=vw)[:, :, :, hd], 1.0)
            # v bias add (zero in practice): bv broadcast over partitions and st
            bv_bc = sb.tile([P, hl * vw], F32)
            nc.gpsimd.partition_broadcast(bv_bc, bv_sb)
            nc.vector.tensor_add(
                v_sb, v_sb,
                bv_bc[:, :].rearrange("p (o e) -> p o e", o=1)
                .broadcast_to([P, nst, hl * vw]))

            # ---- phase A: attention, one head PAIR at a time ----
            # Heads 2p (partitions 0-63) and 2p+1 (64-127) issue adjacent
            # logits matmuls into disjoint PE row groups -> they execute
            # concurrently. One ACT exp op covers both heads per sk tile.
            # exp lives in two sk-half tiles (bufs=4) so iteration i+2's exp
            # stream only waits on the oldest half's normalize, not a whole
            # iteration.
            vals_sb = sb.tile([hd, hl, s], BF16)
            hst = nst // 2
            cl = ch // P

            def emit_outT(c_):
                # output projection for chunk c_ (inputs were finalized one
                # chunk ago, so nothing here stalls the in-order engines)
                o_sb = sb2.tile([P, d // P, ch], F32, tag="attn", bufs=3,
                                name="o_sb")
                for mt in range(d // P):
                    po = ps.tile([P, ch], F32, tag="acc", name="po")
                    for kt in range(hl):
                        nc.tensor.matmul(
                            po,
                            wo_sb[:, kt, mt * P:(mt + 1) * P],
                            vals_sb[:, kt, c_ * ch:(c_ + 1) * ch],
                            start=(kt == 0), stop=(kt == hl - 1),
                        )
                    nc.scalar.copy(o_sb[:, mt, :], po)
                nc.sync.dma_start(
                    outT[:, :].rearrange("(mt p) x -> p mt x", p=P)
                    [:, :, c_ * ch:(c_ + 1) * ch], o_sb)

            for c in range(nch):
                for pr in range(hp):
                    eh = [None, None]
                    pv0 = ps.tile([vw, ch], F32, tag="acc")
                    pv1 = ps.tile([vw, ch], F32, tag="acc2")
                    pv = (pv0, pv1)

                    def vals_mm(skt_):
                        cur_ = eh[skt_ // hst]
                        for hh in range(2):
                            h_ = 2 * pr + hh
                            nc.tensor.matmul(
                                pv[hh],
                                v_sb[:, skt_, h_ * vw:(h_ + 1) * vw],
                                cur_[:, skt_ % hst, hh, :],
                                start=(skt_ == 0), stop=(skt_ == nst - 1),
                            )

                    # software pipeline: vals matmuls run one sk tile behind
                    # the logits/exp, so the in-order PE never waits on ACT.
                    for skt in range(nst):
                        if skt % hst == 0:
                            eh[skt // hst] = sb2.tile([P, hst, 2, ch], BF16,
                                                      tag="exp", bufs=4,
                                                      name="eh")
                        cur = eh[skt // hst]
                        pl = ps.tile([P, 2, ch], F32, tag="l")
                        for hh in range(2):
                            b0 = hh * hd
                            nc.tensor.matmul(
                                pl[:, hh, :],
                                k_sb[b0:b0 + hd, pr, skt * P:(skt + 1) * P],
                                q_sb[b0:b0 + hd, pr, c * ch:(c + 1) * ch],
                                start=True, stop=True,
                            )
                        nc.scalar.activation(cur[:, skt % hst, :, :], pl, EXP,
                                             scale=1.0 / hd)
                        if skt >= 1:
                            vals_mm(skt - 1)
                        if skt == min(6, nst - 2) and pr == 0 and c > 0:
                            emit_outT(c - 1)
                    vals_mm(nst - 1)

                    # Reciprocal chains for both heads, stages interleaved so
                    # the DMA/gpsimd hops of one head hide under DVE work of
                    # the other. (HW partition_broadcast reads physical
                    # partition 0, and the [1, ch] row is spread over 128
                    # partitions for the reciprocal so all DVE lanes work.)
                    sums64 = [None, None]
                    sums_sq = [None, None]
                    recip_sq = [None, None]
                    recip = [None, None]
                    recip_bc = [None, None]
                    for hh in range(2):
                        sums64[hh] = sb2.tile([P, ch], F32, tag="sums64",
                                              bufs=4, name=f"sums64_{hh}")
                        nc.scalar.copy(sums64[hh][hd:hd + 1, :],
                                       pv[hh][hd:hd + 1, :])
                        sums_sq[hh] = sb2.tile([P, cl], F32, tag="sums_sq",
                                               bufs=4, name=f"sums_sq_{hh}")
                        nc.sync.dma_start(sums_sq[hh], sums64[hh][hd:hd + 1, :])
                    for hh in range(2):
                        recip_sq[hh] = sb2.tile([P, cl], F32, tag="recip_sq",
                                                bufs=4, name=f"recip_sq_{hh}")
                        nc.vector.reciprocal(recip_sq[hh], sums_sq[hh])
                        recip[hh] = sb2.tile([1, ch], F32, tag="recip",
                                             bufs=4, name=f"recip_{hh}")
                        nc.sync.dma_start(recip[hh], recip_sq[hh])
                    for hh in range(2):
                        recip_bc[hh] = sb2.tile([P, ch], F32, tag="rbc",
                                                bufs=4, name=f"recip_bc_{hh}")
                        nc.gpsimd.partition_broadcast(recip_bc[hh], recip[hh])
                    for hh in range(2):
                        h = 2 * pr + hh
                        # vals row-normalize fused into the PSUM->SBUF copy
                        nc.vector.tensor_tensor(
                            vals_sb[:, h, c * ch:(c + 1) * ch],
                            pv[hh][0:hd, :], recip_bc[hh][0:hd, :],
                            op=mybir.AluOpType.mult)
                    # attn normalize + store; half-major so the older exp half
                    # frees its slot first
                    for half in range(2):
                        for hh in range(2):
                            h = 2 * pr + hh
                            attn_st = sb2.tile([P, hst, ch], F32, tag="attn",
                                               bufs=3)
                            nc.vector.tensor_tensor(
                                attn_st,
                                eh[half][:, :, hh, :],
                                recip_bc[hh][:, :]
                                .rearrange("p (o n) -> p o n", o=1)
                                .broadcast_to([P, hst, ch]),
                                op=mybir.AluOpType.mult)
                            nc.sync.dma_start(
                                attnT[h].rearrange("(t p) n -> p t n", p=P)
                                [:, half * hst:(half + 1) * hst,
                                 c * ch:(c + 1) * ch],
                                attn_st)

            emit_outT(nch - 1)



    nc.compile()
    return nc


def _get_nc():
    if "full" not in _NC_CACHE:
        _NC_CACHE["full"] = build_nc()
    return _NC_CACHE["full"]


def kernel(x, w_qkv, b_qkv, w_out, b_out):
    global LAST_RESULTS
    _install_ntff_hook()
    x = np.asarray(x, dtype=np.float32)
    w_qkv = np.asarray(w_qkv, dtype=np.float32)
    b_qkv = np.asarray(b_qkv, dtype=np.float32)
    w_out = np.asarray(w_out, dtype=np.float32)
    b_out = np.asarray(b_out, dtype=np.float32)

    # w_qkv rows are per-head interleaved: row h*192+j -> j<64: q, <128: k, <192: v
    wq = np.stack([w_qkv[g * 3 * HD + 0 * HD: g * 3 * HD + 1 * HD] for g in range(H)])   # [H, 64, D]
    wk = np.stack([w_qkv[g * 3 * HD + 1 * HD: g * 3 * HD + 2 * HD] for g in range(H)])
    wv_ = np.stack([w_qkv[g * 3 * HD + 2 * HD: g * 3 * HD + 3 * HD] for g in range(H)])
    bq = np.stack([b_qkv[g * 3 * HD + 0 * HD: g * 3 * HD + 1 * HD] for g in range(H)])   # [H, 64]
    bk = np.stack([b_qkv[g * 3 * HD + 1 * HD: g * 3 * HD + 2 * HD] for g in range(H)])
    bv_ = np.stack([b_qkv[g * 3 * HD + 2 * HD: g * 3 * HD + 3 * HD] for g in range(H)])

    in_maps = []
    for core in range(8):
        b = core // 4
        hg = core % 4
        gh = [hg * HL + i for i in range(HL)]     # global head ids
        xT = np.ascontiguousarray(x[b].T).astype(ml_dtypes.bfloat16)
        # wqk cols: [q_h0|q_h1], [k_h0|k_h1], [q_h2|q_h3], [k_h2|k_h3]
        cols = []
        bqk_cols = []
        for p_ in range(HL // 2):
            h0, h1 = gh[2 * p_], gh[2 * p_ + 1]
            cols.append(np.concatenate([wq[h0], wq[h1]], axis=0))   # [128, D]
            bqk_cols.append(np.concatenate([bq[h0], bq[h1]]))
            cols.append(np.concatenate([wk[h0], wk[h1]], axis=0))
            bqk_cols.append(np.concatenate([bk[h0], bk[h1]]))
        wqk_arr = np.concatenate(cols, axis=0).T                     # [D, 512]
        bqk_arr = np.stack(bqk_cols, axis=1)                         # [128, 4] (mt order)
        wv_arr = np.concatenate([wv_[g] for g in gh], axis=0).T      # [D, 256]
        bv_arr = np.zeros((1, HL * (HD + 1)), np.float32)
        for i, g in enumerate(gh):
            bv_arr[0, i * (HD + 1): i * (HD + 1) + HD] = bv_[g]
        wo_arr = np.ascontiguousarray(
            w_out[:, gh[0] * HD:(gh[-1] + 1) * HD].T)                # [256, D]
        in_maps.append({
            "xT": xT,
            "wqk": np.ascontiguousarray(wqk_arr).astype(ml_dtypes.bfloat16),
            "wv": np.ascontiguousarray(wv_arr).astype(ml_dtypes.bfloat16),
            "wo": wo_arr.astype(ml_dtypes.bfloat16),
            "bqk": np.ascontiguousarray(bqk_arr),
            "bv": bv_arr,
        })

    nc = _get_nc()
    res = run_bass_kernel_spmd(nc, in_maps, core_ids=list(range(8)))
    LAST_RESULTS = res

    attn = np.empty((B, H, S, S), np.float32)
    out = np.zeros((B, S, D), np.float32)
    for core in range(8):
        b = core // 4
        hg = core % 4
        r = res.results[core]
        for i in range(HL):
            attn[b, hg * HL + i] = r["attnT"][i].T
        out[b] += r["outT"].T
    out += b_out
    return out, attn


# revision 23
# speedup vs baseline: 1.0254x; 1.0141x over previous
"""Multi-head attention (B=2, S=2048, D=1024, H=16, HD=64) on 8 TRN2 NeuronCores.

Sharding: data-parallel over batch (2) x tensor-parallel over head groups (4).
Core c handles batch b = c // 4, local heads hg*4 .. hg*4+3 where hg = c % 4.

Per-core device flow (all matmuls bf16, accumulation fp32 in PSUM):
  Phase Q: qT/kT = (x W_q/k^T)^T via PE (contraction over D), v = x W_v^T.
           q/k stored transposed ([hd, s], head pairs packed at partitions
           0-63 / 64-127); v stored [s, hd] with a ones column appended.
  Phase A: per (head, sq-chunk of 512):
           logitsT[sk, sq] = k^T q on PE (K=hd=64),
           exp via ACT (scale=1/64 folded, PSUM->SBUF, bf16),
           valsT[hd+1, sq] = [v|1]^T exp accumulated over sk tiles
             (row 64 = softmax denominator),
           recip via DVE, broadcast via GPSIMD,
           attnT = exp * recip (DVE) -> DMA to HBM,
           vals row-normalize fused into the PSUM->SBUF copy.
  Phase O: outT[d, s] = W_o^T vals^T on PE (K=64 per head block) -> HBM.

Host: shards/prepacks weights per core, runs SPMD on 8 cores, transposes
attnT/outT shards back and sums the 4 partial outT per batch.
"""
import os
import sys
import types

import numpy as np
import ml_dtypes

import concourse.bacc as bacc
import concourse.mybir as mybir
import concourse.tile as tile
from concourse.bass_utils import run_bass_kernel_spmd

F32 = mybir.dt.float32
BF16 = mybir.dt.bfloat16
EXP = mybir.ActivationFunctionType.Exp

B, S, D = 2, 2048, 1024
H, HD = 16, 64
HL = 4          # heads per core
CH = 512        # sq chunk
P = 128

# Results of the last SPMD run (exec_time_ns etc.), for the test harness.
LAST_RESULTS = None
_NC_CACHE = {}


def _install_ntff_hook():
    """The agent image's antenv lacks axon_hooks; inject it so trace=True
    (BASS_TRACE=1) can capture NTFF profiles under axon."""
    try:
        import antenv
        if "antenv.axon_hooks" in sys.modules:
            return
        mod = types.ModuleType("antenv.axon_hooks")
        mod._hook = None

        def set_axon_ntff_profile_hook(h):
            mod._hook = h

        def get_axon_ntff_profile_hook():
            return mod._hook

        mod.set_axon_ntff_profile_hook = set_axon_ntff_profile_hook
        mod.get_axon_ntff_profile_hook = get_axon_ntff_profile_hook
        sys.modules["antenv.axon_hooks"] = mod
        antenv.axon_hooks = mod
        from trn_agent_boot.trn_boot import _ntff_profile_via_ctypes
        hook = _ntff_profile_via_ctypes('/opt/axon/libaxon_pjrt.so')
        if hook is not None:
            set_axon_ntff_profile_hook(hook)
    except Exception:
        pass


def build_nc(s=S, d=D, hl=HL, hd=HD, ch=CH):
    """Build the per-core Bass program. Parameterized so a small config can be
    checked in CoreSim; the full size is (2048, 1024, 4, 64, 512)."""
    nkt = d // P                  # k-tiles over model dim
    nst = s // P                  # s tiles (also sk tiles)
    nch = s // ch                 # sq chunks
    hp = hl // 2                  # head pairs
    vw = hd + 1                   # v block width incl. ones column
    blk = 2                       # sk tiles per logits/exp block

    nc = bacc.Bacc(None, target_bir_lowering=False)

    xT = nc.dram_tensor("xT", [d, s], BF16, kind="ExternalInput")
    # wqk M-tiles of 128 cols: [q_h0|q_h1], [k_h0|k_h1], [q_h2|q_h3], [k_h2|k_h3]
    wqk = nc.dram_tensor("wqk", [d, hl * 2 * hd], BF16, kind="ExternalInput")
    wv = nc.dram_tensor("wv", [d, hl * hd], BF16, kind="ExternalInput")
    wo = nc.dram_tensor("wo", [hl * hd, d], BF16, kind="ExternalInput")
    bqk = nc.dram_tensor("bqk", [P, hl], F32, kind="ExternalInput")
    bv = nc.dram_tensor("bv", [1, hl * vw], F32, kind="ExternalInput")
    attnT = nc.dram_tensor("attnT", [hl, s, s], F32, kind="ExternalOutput")
    outT = nc.dram_tensor("outT", [d, s], F32, kind="ExternalOutput")

    with tile.TileContext(nc) as tc:
        with (
            tc.tile_pool(name="sb", bufs=1) as sb,
            tc.tile_pool(name="sb2", bufs=2) as sb2,
            tc.tile_pool(name="ps", bufs=2, space="PSUM") as ps,
        ):
            # ---- loads ----
            # xT lives in two 16KB tiles sharing the "exp" tag/slots with
            # phase A's exp half-tiles (4 slots of 16KB)
            hkt = nkt // 2
            xT_a = sb2.tile([P, hkt, s], BF16, tag="exp", bufs=4)
            nc.sync.dma_start(
                xT_a, xT[0:hkt * P, :].rearrange("(kt p) s -> p kt s", p=P))
            xT_b = sb2.tile([P, nkt - hkt, s], BF16, tag="exp", bufs=4)
            nc.sync.dma_start(
                xT_b, xT[hkt * P:, :].rearrange("(kt p) s -> p kt s", p=P))
            xT_parts = (xT_a, xT_b)

            def xT_t(kt):
                return xT_parts[kt // hkt][:, kt % hkt, :]
            wqk_sb = sb.tile([P, nkt, hl * 2 * hd], BF16)
            nc.sync.dma_start(wqk_sb, wqk[:, :].rearrange("(kt p) e -> p kt e", p=P))
            wv_sb = sb.tile([P, nkt, hl * hd], BF16)
            nc.sync.dma_start(wv_sb, wv[:, :].rearrange("(kt p) e -> p kt e", p=P))
            wo_sb = sb.tile([hd, hl, d], BF16)
            nc.sync.dma_start(wo_sb, wo[:, :].rearrange("(kt p) e -> p kt e", p=hd))
            bqk_sb = sb.tile([P, hl], F32)
            nc.sync.dma_start(bqk_sb, bqk[:, :])
            bv_sb = sb.tile([1, hl * vw], F32)
            nc.sync.dma_start(bv_sb, bv[:, :])

            # ---- phase Q: projections ----
            q_sb = sb.tile([P, hp, s], BF16)      # head pair p: h=2p at part 0-63, h=2p+1 at 64-127
            k_sb = sb.tile([P, hp, s], BF16)
            for mt in range(2 * hp):              # [qp0, kp0, qp1, kp1]
                dst = q_sb if mt % 2 == 0 else k_sb
                for c in range(nch):
                    pq = ps.tile([P, ch], F32, tag="acc")
                    for kt in range(nkt):
                        nc.tensor.matmul(
                            pq,
                            wqk_sb[:, kt, mt * P:(mt + 1) * P],
                            xT_t(kt)[:, c * ch:(c + 1) * ch],
                            start=(kt == 0), stop=(kt == nkt - 1),
                        )
                    # copy + per-partition bias add (bias is zero in practice)
                    nc.vector.tensor_scalar_add(
                        dst[:, mt // 2, c * ch:(c + 1) * ch], pq,
                        bqk_sb[:, mt:mt + 1])
            v_sb = sb.tile([P, nst, hl * vw], BF16)
            for st in range(nst):
                pv = ps.tile([P, ch], F32, tag="acc")
                for kt in range(nkt):
                    nc.tensor.matmul(
                        pv[:, 0:hl * hd],
                        xT_t(kt)[:, st * P:(st + 1) * P],
                        wv_sb[:, kt, :],
                        start=(kt == 0), stop=(kt == nkt - 1),
                    )
                nc.vector.tensor_copy(
                    v_sb[:, st, :].rearrange("p (h e) -> p h e", e=vw)[:, :, 0:hd],
                    pv[:, 0:hl * hd].rearrange("p (h e) -> p h e", e=hd),
                )
            nc.vector.memset(
                v_sb[:, :, :].rearrange("p st (h e) -> p st h e", e=vw)[:, :, :, hd], 1.0)
            # v bias add (zero in practice): bv broadcast over partitions and st
            bv_bc = sb.tile([P, hl * vw], F32)
            nc.gpsimd.partition_broadcast(bv_bc, bv_sb)
            nc.vector.tensor_add(
                v_sb, v_sb,
                bv_bc[:, :].rearrange("p (o e) -> p o e", o=1)
                .broadcast_to([P, nst, hl * vw]))

            # ---- phase A: attention, one head PAIR at a time ----
            # Heads 2p (partitions 0-63) and 2p+1 (64-127) issue adjacent
            # logits matmuls into disjoint PE row groups -> they execute
            # concurrently. One ACT exp op covers both heads per sk tile.
            # exp lives in two sk-half tiles (bufs=4) so iteration i+2's exp
            # stream only waits on the oldest half's normalize, not a whole
            # iteration.
            vals_sb = sb.tile([hd, hl, s], BF16)
            hst = nst // 2
            cl = ch // P

            def emit_outT(c_):
                # output projection for chunk c_ (inputs were finalized one
                # chunk ago, so nothing here stalls the in-order engines)
                o_sb = sb2.tile([P, d // P, ch], F32, tag="attn", bufs=3,
                                name="o_sb")
                for mt in range(d // P):
                    po = ps.tile([P, ch], F32, tag="acc", name="po")
                    for kt in range(hl):
                        nc.tensor.matmul(
                            po,
                            wo_sb[:, kt, mt * P:(mt + 1) * P],
                            vals_sb[:, kt, c_ * ch:(c_ + 1) * ch],
                            start=(kt == 0), stop=(kt == hl - 1),
                        )
                    nc.scalar.copy(o_sb[:, mt, :], po)
                nc.sync.dma_start(
                    outT[:, :].rearrange("(mt p) x -> p mt x", p=P)
                    [:, :, c_ * ch:(c_ + 1) * ch], o_sb)

            for c in range(nch):
                for pr in range(hp):
                    eh = [None, None]
                    pv0 = ps.tile([vw, ch], F32, tag="acc")
                    pv1 = ps.tile([vw, ch], F32, tag="acc2")
                    pv = (pv0, pv1)

                    def vals_mm(skt_):
                        cur_ = eh[skt_ // hst]
                        for hh in range(2):
                            h_ = 2 * pr + hh
                            nc.tensor.matmul(
                                pv[hh],
                                v_sb[:, skt_, h_ * vw:(h_ + 1) * vw],
                                cur_[:, skt_ % hst, hh, :],
                                start=(skt_ == 0), stop=(skt_ == nst - 1),
                            )

                    # software pipeline: vals matmuls run one sk tile behind
                    # the logits/exp, so the in-order PE never waits on ACT.
                    for skt in range(nst):
                        if skt % hst == 0:
                            eh[skt // hst] = sb2.tile([P, hst, 2, ch], BF16,
                                                      tag="exp", bufs=4,
                                                      name="eh")
                        cur = eh[skt // hst]
                        pl = ps.tile([P, 2, ch], F32, tag="l")
                        for hh in range(2):
                            b0 = hh * hd
                            nc.tensor.matmul(
                                pl[:, hh, :],
                                k_sb[b0:b0 + hd, pr, skt * P:(skt + 1) * P],
                                q_sb[b0:b0 + hd, pr, c * ch:(c + 1) * ch],
                                start=True, stop=True,
                            )
                        nc.scalar.activation(cur[:, skt % hst, :, :], pl, EXP,
                                             scale=1.0 / hd)
                        if skt >= 1:
                            vals_mm(skt - 1)
                        if skt == min(6, nst - 2) and pr == 0 and c > 0:
                            emit_outT(c - 1)
                    vals_mm(nst - 1)

                    # Reciprocal chains for both heads, stages interleaved so
                    # the DMA/gpsimd hops of one head hide under DVE work of
                    # the other. (HW partition_broadcast reads physical
                    # partition 0, and the [1, ch] row is spread over 128
                    # partitions for the reciprocal so all DVE lanes work.)
                    sums64 = [None, None]
                    sums_sq = [None, None]
                    recip_sq = [None, None]
                    recip = [None, None]
                    recip_bc = [None, None]
                    for hh in range(2):
                        sums64[hh] = sb2.tile([P, ch], F32, tag="sums64",
                                              bufs=4, name=f"sums64_{hh}")
                        nc.scalar.copy(sums64[hh][hd:hd + 1, :],
                                       pv[hh][hd:hd + 1, :])
                        sums_sq[hh] = sb2.tile([P, cl], F32, tag="sums_sq",
                                               bufs=4, name=f"sums_sq_{hh}")
                        nc.gpsimd.dma_start(sums_sq[hh], sums64[hh][hd:hd + 1, :])
                    for hh in range(2):
                        recip_sq[hh] = sb2.tile([P, cl], F32, tag="recip_sq",
                                                bufs=4, name=f"recip_sq_{hh}")
                        nc.vector.reciprocal(recip_sq[hh], sums_sq[hh])
                        recip[hh] = sb2.tile([1, ch], F32, tag="recip",
                                             bufs=4, name=f"recip_{hh}")
                        nc.gpsimd.dma_start(recip[hh], recip_sq[hh])
                    for hh in range(2):
                        recip_bc[hh] = sb2.tile([P, ch], F32, tag="rbc",
                                                bufs=4, name=f"recip_bc_{hh}")
                        nc.gpsimd.partition_broadcast(recip_bc[hh], recip[hh])
                    for hh in range(2):
                        h = 2 * pr + hh
                        # vals row-normalize fused into the PSUM->SBUF copy
                        nc.vector.tensor_tensor(
                            vals_sb[:, h, c * ch:(c + 1) * ch],
                            pv[hh][0:hd, :], recip_bc[hh][0:hd, :],
                            op=mybir.AluOpType.mult)
                    # attn normalize + store; half-major so the older exp half
                    # frees its slot first
                    for half in range(2):
                        for hh in range(2):
                            h = 2 * pr + hh
                            attn_st = sb2.tile([P, hst, ch], F32, tag="attn",
                                               bufs=3)
                            nc.vector.tensor_tensor(
                                attn_st,
                                eh[half][:, :, hh, :],
                                recip_bc[hh][:, :]
                                .rearrange("p (o n) -> p o n", o=1)
                                .broadcast_to([P, hst, ch]),
                                op=mybir.AluOpType.mult)
                            nc.sync.dma_start(
                                attnT[h].rearrange("(t p) n -> p t n", p=P)
                                [:, half * hst:(half + 1) * hst,
                                 c * ch:(c + 1) * ch],
                                attn_st)

            emit_outT(nch - 1)



    nc.compile()
    return nc


def _get_nc():
    if "full" not in _NC_CACHE:
        _NC_CACHE["full"] = build_nc()
    return _NC_CACHE["full"]


def kernel(x, w_qkv, b_qkv, w_out, b_out):
    global LAST_RESULTS
    _install_ntff_hook()
    x = np.asarray(x, dtype=np.float32)
    w_qkv = np.asarray(w_qkv, dtype=np.float32)
    b_qkv = np.asarray(b_qkv, dtype=np.float32)
    w_out = np.asarray(w_out, dtype=np.float32)
    b_out = np.asarray(b_out, dtype=np.float32)

    # w_qkv rows are per-head interleaved: row h*192+j -> j<64: q, <128: k, <192: v
    wq = np.stack([w_qkv[g * 3 * HD + 0 * HD: g * 3 * HD + 1 * HD] for g in range(H)])   # [H, 64, D]
    wk = np.stack([w_qkv[g * 3 * HD + 1 * HD: g * 3 * HD + 2 * HD] for g in range(H)])
    wv_ = np.stack([w_qkv[g * 3 * HD + 2 * HD: g * 3 * HD + 3 * HD] for g in range(H)])
    bq = np.stack([b_qkv[g * 3 * HD + 0 * HD: g * 3 * HD + 1 * HD] for g in range(H)])   # [H, 64]
    bk = np.stack([b_qkv[g * 3 * HD + 1 * HD: g * 3 * HD + 2 * HD] for g in range(H)])
    bv_ = np.stack([b_qkv[g * 3 * HD + 2 * HD: g * 3 * HD + 3 * HD] for g in range(H)])

    in_maps = []
    for core in range(8):
        b = core // 4
        hg = core % 4
        gh = [hg * HL + i for i in range(HL)]     # global head ids
        xT = np.ascontiguousarray(x[b].T).astype(ml_dtypes.bfloat16)
        # wqk cols: [q_h0|q_h1], [k_h0|k_h1], [q_h2|q_h3], [k_h2|k_h3]
        cols = []
        bqk_cols = []
        for p_ in range(HL // 2):
            h0, h1 = gh[2 * p_], gh[2 * p_ + 1]
            cols.append(np.concatenate([wq[h0], wq[h1]], axis=0))   # [128, D]
            bqk_cols.append(np.concatenate([bq[h0], bq[h1]]))
            cols.append(np.concatenate([wk[h0], wk[h1]], axis=0))
            bqk_cols.append(np.concatenate([bk[h0], bk[h1]]))
        wqk_arr = np.concatenate(cols, axis=0).T                     # [D, 512]
        bqk_arr = np.stack(bqk_cols, axis=1)                         # [128, 4] (mt order)
        wv_arr = np.concatenate([wv_[g] for g in gh], axis=0).T      # [D, 256]
        bv_arr = np.zeros((1, HL * (HD + 1)), np.float32)
        for i, g in enumerate(gh):
            bv_arr[0, i * (HD + 1): i * (HD + 1) + HD] = bv_[g]
        wo_arr = np.ascontiguousarray(
            w_out[:, gh[0] * HD:(gh[-1] + 1) * HD].T)                # [256, D]
        in_maps.append({
            "xT": xT,
            "wqk": np.ascontiguousarray(wqk_arr).astype(ml_dtypes.bfloat16),
            "wv": np.ascontiguousarray(wv_arr).astype(ml_dtypes.bfloat16),
            "wo": wo_arr.astype(ml_dtypes.bfloat16),
            "bqk": np.ascontiguousarray(bqk_arr),
            "bv": bv_arr,
        })

    nc = _get_nc()
    res = run_bass_kernel_spmd(nc, in_maps, core_ids=list(range(8)))
    LAST_RESULTS = res

    attn = np.empty((B, H, S, S), np.float32)
    out = np.zeros((B, S, D), np.float32)
    for core in range(8):
        b = core // 4
        hg = core % 4
        r = res.results[core]
        for i in range(HL):
            attn[b, hg * HL + i] = r["attnT"][i].T
        out[b] += r["outT"].T
    out += b_out
    return out, attn


# revision 24
# speedup vs baseline: 1.0427x; 1.0169x over previous
"""Multi-head attention (B=2, S=2048, D=1024, H=16, HD=64) on 8 TRN2 NeuronCores.

Sharding: data-parallel over batch (2) x tensor-parallel over head groups (4).
Core c handles batch b = c // 4, local heads hg*4 .. hg*4+3 where hg = c % 4.

Per-core device flow (all matmuls bf16, accumulation fp32 in PSUM):
  Phase Q: qT/kT = (x W_q/k^T)^T via PE (contraction over D), v = x W_v^T.
           q/k stored transposed ([hd, s], head pairs packed at partitions
           0-63 / 64-127); v stored [s, hd] with a ones column appended.
  Phase A: per (head, sq-chunk of 512):
           logitsT[sk, sq] = k^T q on PE (K=hd=64),
           exp via ACT (scale=1/64 folded, PSUM->SBUF, bf16),
           valsT[hd+1, sq] = [v|1]^T exp accumulated over sk tiles
             (row 64 = softmax denominator),
           recip via DVE, broadcast via GPSIMD,
           attnT = exp * recip (DVE) -> DMA to HBM,
           vals row-normalize fused into the PSUM->SBUF copy.
  Phase O: outT[d, s] = W_o^T vals^T on PE (K=64 per head block) -> HBM.

Host: shards/prepacks weights per core, runs SPMD on 8 cores, transposes
attnT/outT shards back and sums the 4 partial outT per batch.
"""
import os
import sys
import types

import numpy as np
import ml_dtypes

import concourse.bacc as bacc
import concourse.mybir as mybir
import concourse.tile as tile
from concourse.bass_utils import run_bass_kernel_spmd

F32 = mybir.dt.float32
BF16 = mybir.dt.bfloat16
EXP = mybir.ActivationFunctionType.Exp

B, S, D = 2, 2048, 1024
H, HD = 16, 64
HL = 4          # heads per core
CH = 512        # sq chunk
P = 128

# Results of the last SPMD run (exec_time_ns etc.), for the test harness.
LAST_RESULTS = None
_NC_CACHE = {}


def _install_ntff_hook():
    """The agent image's antenv lacks axon_hooks; inject it so trace=True
    (BASS_TRACE=1) can capture NTFF profiles under axon."""
    try:
        import antenv
        if "antenv.axon_hooks" in sys.modules:
            return
        mod = types.ModuleType("antenv.axon_hooks")
        mod._hook = None

        def set_axon_ntff_profile_hook(h):
            mod._hook = h

        def get_axon_ntff_profile_hook():
            return mod._hook

        mod.set_axon_ntff_profile_hook = set_axon_ntff_profile_hook
        mod.get_axon_ntff_profile_hook = get_axon_ntff_profile_hook
        sys.modules["antenv.axon_hooks"] = mod
        antenv.axon_hooks = mod
        from trn_agent_boot.trn_boot import _ntff_profile_via_ctypes
        hook = _ntff_profile_via_ctypes('/opt/axon/libaxon_pjrt.so')
        if hook is not None:
            set_axon_ntff_profile_hook(hook)
    except Exception:
        pass


def build_nc(s=S, d=D, hl=HL, hd=HD, ch=CH):
    """Build the per-core Bass program. Parameterized so a small config can be
    checked in CoreSim; the full size is (2048, 1024, 4, 64, 512)."""
    nkt = d // P                  # k-tiles over model dim
    nst = s // P                  # s tiles (also sk tiles)
    nch = s // ch                 # sq chunks
    hp = hl // 2                  # head pairs
    vw = hd + 1                   # v block width incl. ones column
    blk = 2                       # sk tiles per logits/exp block

    nc = bacc.Bacc(None, target_bir_lowering=False)

    xT = nc.dram_tensor("xT", [d, s], BF16, kind="ExternalInput")
    # wqk M-tiles of 128 cols: [q_h0|q_h1], [k_h0|k_h1], [q_h2|q_h3], [k_h2|k_h3]
    wqk = nc.dram_tensor("wqk", [d, hl * 2 * hd], BF16, kind="ExternalInput")
    wv = nc.dram_tensor("wv", [d, hl * hd], BF16, kind="ExternalInput")
    wo = nc.dram_tensor("wo", [hl * hd, d], BF16, kind="ExternalInput")
    bqk = nc.dram_tensor("bqk", [P, hl], F32, kind="ExternalInput")
    bv = nc.dram_tensor("bv", [1, hl * vw], F32, kind="ExternalInput")
    attnT = nc.dram_tensor("attnT", [hl, s, s], F32, kind="ExternalOutput")
    outT = nc.dram_tensor("outT", [d, s], F32, kind="ExternalOutput")

    with tile.TileContext(nc) as tc:
        with (
            tc.tile_pool(name="sb", bufs=1) as sb,
            tc.tile_pool(name="sb2", bufs=2) as sb2,
            tc.tile_pool(name="ps", bufs=2, space="PSUM") as ps,
        ):
            # ---- loads ----
            # xT lives in two 16KB tiles sharing the "exp" tag/slots with
            # phase A's exp half-tiles (4 slots of 16KB)
            hkt = nkt // 2
            xT_a = sb2.tile([P, hkt, s], BF16, tag="exp", bufs=4)
            nc.sync.dma_start(
                xT_a, xT[0:hkt * P, :].rearrange("(kt p) s -> p kt s", p=P))
            xT_b = sb2.tile([P, nkt - hkt, s], BF16, tag="exp", bufs=4)
            nc.sync.dma_start(
                xT_b, xT[hkt * P:, :].rearrange("(kt p) s -> p kt s", p=P))
            xT_parts = (xT_a, xT_b)

            def xT_t(kt):
                return xT_parts[kt // hkt][:, kt % hkt, :]
            wqk_sb = sb.tile([P, nkt, hl * 2 * hd], BF16)
            nc.sync.dma_start(wqk_sb, wqk[:, :].rearrange("(kt p) e -> p kt e", p=P))
            wv_sb = sb.tile([P, nkt, hl * hd], BF16)
            nc.sync.dma_start(wv_sb, wv[:, :].rearrange("(kt p) e -> p kt e", p=P))
            wo_sb = sb.tile([hd, hl, d], BF16)
            nc.sync.dma_start(wo_sb, wo[:, :].rearrange("(kt p) e -> p kt e", p=hd))
            bqk_sb = sb.tile([P, hl], F32)
            nc.sync.dma_start(bqk_sb, bqk[:, :])
            bv_sb = sb.tile([1, hl * vw], F32)
            nc.sync.dma_start(bv_sb, bv[:, :])

            # ---- phase Q: projections ----
            q_sb = sb.tile([P, hp, s], BF16)      # head pair p: h=2p at part 0-63, h=2p+1 at 64-127
            k_sb = sb.tile([P, hp, s], BF16)
            for mt in range(2 * hp):              # [qp0, kp0, qp1, kp1]
                dst = q_sb if mt % 2 == 0 else k_sb
                for c in range(nch):
                    pq = ps.tile([P, ch], F32, tag="acc")
                    for kt in range(nkt):
                        nc.tensor.matmul(
                            pq,
                            wqk_sb[:, kt, mt * P:(mt + 1) * P],
                            xT_t(kt)[:, c * ch:(c + 1) * ch],
                            start=(kt == 0), stop=(kt == nkt - 1),
                        )
                    # copy + per-partition bias add (bias is zero in practice)
                    nc.vector.tensor_scalar_add(
                        dst[:, mt // 2, c * ch:(c + 1) * ch], pq,
                        bqk_sb[:, mt:mt + 1])
            v_sb = sb.tile([P, nst, hl * vw], BF16)
            for st in range(nst):
                pv = ps.tile([P, ch], F32, tag="acc")
                for kt in range(nkt):
                    nc.tensor.matmul(
                        pv[:, 0:hl * hd],
                        xT_t(kt)[:, st * P:(st + 1) * P],
                        wv_sb[:, kt, :],
                        start=(kt == 0), stop=(kt == nkt - 1),
                    )
                nc.vector.tensor_copy(
                    v_sb[:, st, :].rearrange("p (h e) -> p h e", e=vw)[:, :, 0:hd],
                    pv[:, 0:hl * hd].rearrange("p (h e) -> p h e", e=hd),
                )
            nc.vector.memset(
                v_sb[:, :, :].rearrange("p st (h e) -> p st h e", e=vw)[:, :, :, hd], 1.0)
            # v bias add (zero in practice): bv broadcast over partitions and st
            bv_bc = sb.tile([P, hl * vw], F32)
            nc.gpsimd.partition_broadcast(bv_bc, bv_sb)
            nc.vector.tensor_add(
                v_sb, v_sb,
                bv_bc[:, :].rearrange("p (o e) -> p o e", o=1)
                .broadcast_to([P, nst, hl * vw]))

            # ---- phase A: attention, one head PAIR at a time ----
            # Heads 2p (partitions 0-63) and 2p+1 (64-127) issue adjacent
            # logits matmuls into disjoint PE row groups -> they execute
            # concurrently. One ACT exp op covers both heads per sk tile.
            # exp lives in two sk-half tiles (bufs=4) so iteration i+2's exp
            # stream only waits on the oldest half's normalize, not a whole
            # iteration.
            vals_sb = sb.tile([hd, hl, s], BF16)
            hst = nst // 2
            cl = ch // P

            def emit_outT(c_):
                # output projection for chunk c_ (inputs were finalized one
                # chunk ago, so nothing here stalls the in-order engines)
                o_sb = sb2.tile([P, d // P, ch], F32, tag="attn", bufs=3,
                                name="o_sb")
                for mt in range(d // P):
                    po = ps.tile([P, ch], F32, tag="po", bufs=1, name="po")
                    for kt in range(hl):
                        nc.tensor.matmul(
                            po,
                            wo_sb[:, kt, mt * P:(mt + 1) * P],
                            vals_sb[:, kt, c_ * ch:(c_ + 1) * ch],
                            start=(kt == 0), stop=(kt == hl - 1),
                        )
                    nc.scalar.copy(o_sb[:, mt, :], po)
                nc.sync.dma_start(
                    outT[:, :].rearrange("(mt p) x -> p mt x", p=P)
                    [:, :, c_ * ch:(c_ + 1) * ch], o_sb)

            for c in range(nch):
                for pr in range(hp):
                    eh = [None, None]
                    pv0 = ps.tile([vw, ch], F32, tag="acc")
                    pv1 = ps.tile([vw, ch], F32, tag="acc2", bufs=1)
                    pv = (pv0, pv1)

                    def vals_mm(skt_):
                        cur_ = eh[skt_ // hst]
                        for hh in range(2):
                            h_ = 2 * pr + hh
                            nc.tensor.matmul(
                                pv[hh],
                                v_sb[:, skt_, h_ * vw:(h_ + 1) * vw],
                                cur_[:, skt_ % hst, hh, :],
                                start=(skt_ == 0), stop=(skt_ == nst - 1),
                            )

                    # software pipeline: vals matmuls run one sk tile behind
                    # the logits/exp, so the in-order PE never waits on ACT.
                    for skt in range(nst):
                        if skt % hst == 0:
                            eh[skt // hst] = sb2.tile([P, hst, 2, ch], BF16,
                                                      tag="exp", bufs=4,
                                                      name="eh")
                        cur = eh[skt // hst]
                        pl = ps.tile([P, 2, ch], F32, tag="l")
                        for hh in range(2):
                            b0 = hh * hd
                            nc.tensor.matmul(
                                pl[:, hh, :],
                                k_sb[b0:b0 + hd, pr, skt * P:(skt + 1) * P],
                                q_sb[b0:b0 + hd, pr, c * ch:(c + 1) * ch],
                                start=True, stop=True,
                            )
                        nc.scalar.activation(cur[:, skt % hst, :, :], pl, EXP,
                                             scale=1.0 / hd)
                        if skt >= 1:
                            vals_mm(skt - 1)
                        if skt == min(6, nst - 2) and pr == 0 and c > 0:
                            emit_outT(c - 1)
                    vals_mm(nst - 1)

                    # Reciprocal chains for both heads, stages interleaved so
                    # the DMA/gpsimd hops of one head hide under DVE work of
                    # the other. (HW partition_broadcast reads physical
                    # partition 0, and the [1, ch] row is spread over 128
                    # partitions for the reciprocal so all DVE lanes work.)
                    sums64 = [None, None]
                    sums_sq = [None, None]
                    recip_sq = [None, None]
                    recip = [None, None]
                    recip_bc = [None, None]
                    for hh in range(2):
                        sums64[hh] = sb2.tile([P, ch], F32, tag="sums64",
                                              bufs=4, name=f"sums64_{hh}")
                        nc.scalar.copy(sums64[hh][hd:hd + 1, :],
                                       pv[hh][hd:hd + 1, :])
                        sums_sq[hh] = sb2.tile([P, cl], F32, tag="sums_sq",
                                               bufs=4, name=f"sums_sq_{hh}")
                        nc.gpsimd.dma_start(sums_sq[hh], sums64[hh][hd:hd + 1, :])
                    for hh in range(2):
                        recip_sq[hh] = sb2.tile([P, cl], F32, tag="recip_sq",
                                                bufs=4, name=f"recip_sq_{hh}")
                        nc.vector.reciprocal(recip_sq[hh], sums_sq[hh])
                        recip[hh] = sb2.tile([1, ch], F32, tag="recip",
                                             bufs=4, name=f"recip_{hh}")
                        nc.gpsimd.dma_start(recip[hh], recip_sq[hh])
                    for hh in range(2):
                        recip_bc[hh] = sb2.tile([P, ch], F32, tag="rbc",
                                                bufs=4, name=f"recip_bc_{hh}")
                        nc.gpsimd.partition_broadcast(recip_bc[hh], recip[hh])
                    for hh in range(2):
                        h = 2 * pr + hh
                        # vals row-normalize fused into the PSUM->SBUF copy
                        nc.vector.tensor_tensor(
                            vals_sb[:, h, c * ch:(c + 1) * ch],
                            pv[hh][0:hd, :], recip_bc[hh][0:hd, :],
                            op=mybir.AluOpType.mult)
                    # attn normalize + store; half-major so the older exp half
                    # frees its slot first
                    for half in range(2):
                        for hh in range(2):
                            h = 2 * pr + hh
                            attn_st = sb2.tile([P, hst, ch], F32, tag="attn",
                                               bufs=3)
                            nc.vector.tensor_tensor(
                                attn_st,
                                eh[half][:, :, hh, :],
                                recip_bc[hh][:, :]
                                .rearrange("p (o n) -> p o n", o=1)
                                .broadcast_to([P, hst, ch]),
                                op=mybir.AluOpType.mult)
                            nc.sync.dma_start(
                                attnT[h].rearrange("(t p) n -> p t n", p=P)
                                [:, half * hst:(half + 1) * hst,
                                 c * ch:(c + 1) * ch],
                                attn_st)

            emit_outT(nch - 1)



    nc.compile()
    return nc


def _get_nc():
    if "full" not in _NC_CACHE:
        _NC_CACHE["full"] = build_nc()
    return _NC_CACHE["full"]


def kernel(x, w_qkv, b_qkv, w_out, b_out):
    global LAST_RESULTS
    _install_ntff_hook()
    x = np.asarray(x, dtype=np.float32)
    w_qkv = np.asarray(w_qkv, dtype=np.float32)
    b_qkv = np.asarray(b_qkv, dtype=np.float32)
    w_out = np.asarray(w_out, dtype=np.float32)
    b_out = np.asarray(b_out, dtype=np.float32)

    # w_qkv rows are per-head interleaved: row h*192+j -> j<64: q, <128: k, <192: v
    wq = np.stack([w_qkv[g * 3 * HD + 0 * HD: g * 3 * HD + 1 * HD] for g in range(H)])   # [H, 64, D]
    wk = np.stack([w_qkv[g * 3 * HD + 1 * HD: g * 3 * HD + 2 * HD] for g in range(H)])
    wv_ = np.stack([w_qkv[g * 3 * HD + 2 * HD: g * 3 * HD + 3 * HD] for g in range(H)])
    bq = np.stack([b_qkv[g * 3 * HD + 0 * HD: g * 3 * HD + 1 * HD] for g in range(H)])   # [H, 64]
    bk = np.stack([b_qkv[g * 3 * HD + 1 * HD: g * 3 * HD + 2 * HD] for g in range(H)])
    bv_ = np.stack([b_qkv[g * 3 * HD + 2 * HD: g * 3 * HD + 3 * HD] for g in range(H)])

    in_maps = []
    for core in range(8):
        b = core // 4
        hg = core % 4
        gh = [hg * HL + i for i in range(HL)]     # global head ids
        xT = np.ascontiguousarray(x[b].T).astype(ml_dtypes.bfloat16)
        # wqk cols: [q_h0|q_h1], [k_h0|k_h1], [q_h2|q_h3], [k_h2|k_h3]
        cols = []
        bqk_cols = []
        for p_ in range(HL // 2):
            h0, h1 = gh[2 * p_], gh[2 * p_ + 1]
            cols.append(np.concatenate([wq[h0], wq[h1]], axis=0))   # [128, D]
            bqk_cols.append(np.concatenate([bq[h0], bq[h1]]))
            cols.append(np.concatenate([wk[h0], wk[h1]], axis=0))
            bqk_cols.append(np.concatenate([bk[h0], bk[h1]]))
        wqk_arr = np.concatenate(cols, axis=0).T                     # [D, 512]
        bqk_arr = np.stack(bqk_cols, axis=1)                         # [128, 4] (mt order)
        wv_arr = np.concatenate([wv_[g] for g in gh], axis=0).T      # [D, 256]
        bv_arr = np.zeros((1, HL * (HD + 1)), np.float32)
        for i, g in enumerate(gh):
            bv_arr[0, i * (HD + 1): i * (HD + 1) + HD] = bv_[g]
        wo_arr = np.ascontiguousarray(
            w_out[:, gh[0] * HD:(gh[-1] + 1) * HD].T)                # [256, D]
        in_maps.append({
            "xT": xT,
            "wqk": np.ascontiguousarray(wqk_arr).astype(ml_dtypes.bfloat16),
            "wv": np.ascontiguousarray(wv_arr).astype(ml_dtypes.bfloat16),
            "wo": wo_arr.astype(ml_dtypes.bfloat16),
            "bqk": np.ascontiguousarray(bqk_arr),
            "bv": bv_arr,
        })

    nc = _get_nc()
    res = run_bass_kernel_spmd(nc, in_maps, core_ids=list(range(8)))
    LAST_RESULTS = res

    attn = np.empty((B, H, S, S), np.float32)
    out = np.zeros((B, S, D), np.float32)
    for core in range(8):
        b = core // 4
        hg = core % 4
        r = res.results[core]
        for i in range(HL):
            attn[b, hg * HL + i] = r["attnT"][i].T
        out[b] += r["outT"].T
    out += b_out
    return out, attn


# revision 27
# speedup vs baseline: 1.0623x; 1.0187x over previous
"""Multi-head attention (B=2, S=2048, D=1024, H=16, HD=64) on 8 TRN2 NeuronCores.

Sharding: data-parallel over batch (2) x tensor-parallel over head groups (4).
Core c handles batch b = c // 4, local heads hg*4 .. hg*4+3 where hg = c % 4.

Per-core device flow (all matmuls bf16, accumulation fp32 in PSUM):
  Phase Q: qT/kT = (x W_q/k^T)^T via PE (contraction over D), v = x W_v^T.
           q/k stored transposed ([hd, s]); the two heads of a pair sit at
           partitions 0-63 / 64-127 so their logits matmuls run concurrently
           in disjoint PE row groups. v stored [s, hd] + a ones column.
  Phase A: per (sq-chunk of 512, head pair):
           logitsT[sk, sq] = k^T q on PE (K=64, two heads row-tiled),
           exp via one ACT op per sk tile covering both heads (scale=1/64
           folded, PSUM->SBUF, bf16, into two sk-half tiles with 4 slots),
           valsT[hd+1, sq] = [v|1]^T exp accumulated over sk tiles
             (row 64 = softmax denominator),
           reciprocal: denominator row spread over 128 partitions via a
             small gpsimd DMA (full-lane DVE reciprocal), brought back and
             partition-broadcast (HW broadcast reads physical partition 0),
           attnT = exp * recip on DVE -> DMA to HBM,
           vals row-normalize fused into the PSUM->SBUF copy,
           output projection for the previous chunk spread one d-tile per
             sk tile through the stream (PE slack, copies on ACT).

Host: shards/prepacks weights per core, runs SPMD on 8 cores, transposes
attnT/outT shards back and sums the 4 partial outT per batch.
"""
import os
import sys
import types

import numpy as np
import ml_dtypes

import concourse.bacc as bacc
import concourse.mybir as mybir
import concourse.tile as tile
from concourse.bass_utils import run_bass_kernel_spmd

F32 = mybir.dt.float32
BF16 = mybir.dt.bfloat16
EXP = mybir.ActivationFunctionType.Exp

B, S, D = 2, 2048, 1024
H, HD = 16, 64
HL = 4          # heads per core
CH = 512        # sq chunk
P = 128

# Results of the last SPMD run (exec_time_ns etc.), for the test harness.
LAST_RESULTS = None
_NC_CACHE = {}


def _install_ntff_hook():
    """The agent image's antenv lacks axon_hooks; inject it so trace=True
    (BASS_TRACE=1) can capture NTFF profiles under axon."""
    try:
        import antenv
        if "antenv.axon_hooks" in sys.modules:
            return
        mod = types.ModuleType("antenv.axon_hooks")
        mod._hook = None

        def set_axon_ntff_profile_hook(h):
            mod._hook = h

        def get_axon_ntff_profile_hook():
            return mod._hook

        mod.set_axon_ntff_profile_hook = set_axon_ntff_profile_hook
        mod.get_axon_ntff_profile_hook = get_axon_ntff_profile_hook
        sys.modules["antenv.axon_hooks"] = mod
        antenv.axon_hooks = mod
        from trn_agent_boot.trn_boot import _ntff_profile_via_ctypes
        hook = _ntff_profile_via_ctypes('/opt/axon/libaxon_pjrt.so')
        if hook is not None:
            set_axon_ntff_profile_hook(hook)
    except Exception:
        pass


def build_nc(s=S, d=D, hl=HL, hd=HD, ch=CH):
    """Build the per-core Bass program. Parameterized so a small config can be
    checked in CoreSim; the full size is (2048, 1024, 4, 64, 512)."""
    nkt = d // P                  # k-tiles over model dim
    nst = s // P                  # s tiles (also sk tiles)
    nch = s // ch                 # sq chunks
    hp = hl // 2                  # head pairs
    vw = hd + 1                   # v block width incl. ones column
    nmt = d // P                  # out-projection d tiles
    hst = nst // 2                # sk tiles per exp half-tile
    cl = ch // P

    nc = bacc.Bacc(None, target_bir_lowering=False)

    xT = nc.dram_tensor("xT", [d, s], BF16, kind="ExternalInput")
    # wqk col blocks of 128: [q_h0|q_h1], [k_h0|k_h1], [q_h2|q_h3], [k_h2|k_h3]
    wqk = nc.dram_tensor("wqk", [d, hl * 2 * hd], BF16, kind="ExternalInput")
    wv = nc.dram_tensor("wv", [d, hl * hd], BF16, kind="ExternalInput")
    wo = nc.dram_tensor("wo", [hl * hd, d], BF16, kind="ExternalInput")
    bqk = nc.dram_tensor("bqk", [P, hl], F32, kind="ExternalInput")
    bv = nc.dram_tensor("bv", [1, hl * vw], F32, kind="ExternalInput")
    attnT = nc.dram_tensor("attnT", [hl, s, s], F32, kind="ExternalOutput")
    outT = nc.dram_tensor("outT", [d, s], F32, kind="ExternalOutput")

    with tile.TileContext(nc) as tc:
        with (
            tc.tile_pool(name="sb", bufs=1) as sb,
            tc.tile_pool(name="sb2", bufs=2) as sb2,
            tc.tile_pool(name="ps", bufs=2, space="PSUM") as ps,
        ):
            # ---- loads ----
            # xT lives in two 16KB tiles sharing the "exp" tag/slots with
            # phase A's exp half-tiles (4 slots of 16KB)
            hkt = nkt // 2
            xT_a = sb2.tile([P, hkt, s], BF16, tag="exp", bufs=4)
            nc.sync.dma_start(
                xT_a, xT[0:hkt * P, :].rearrange("(kt p) s -> p kt s", p=P))
            xT_b = sb2.tile([P, nkt - hkt, s], BF16, tag="exp", bufs=4)
            nc.sync.dma_start(
                xT_b, xT[hkt * P:, :].rearrange("(kt p) s -> p kt s", p=P))
            xT_parts = (xT_a, xT_b)

            def xT_t(kt):
                return xT_parts[kt // hkt][:, kt % hkt, :]

            wqk_sb = sb.tile([P, nkt, hl * 2 * hd], BF16)
            nc.sync.dma_start(wqk_sb, wqk[:, :].rearrange("(kt p) e -> p kt e", p=P))
            wv_sb = sb.tile([P, nkt, hl * hd], BF16)
            nc.sync.dma_start(wv_sb, wv[:, :].rearrange("(kt p) e -> p kt e", p=P))
            wo_sb = sb.tile([hd, hl, d], BF16)
            nc.sync.dma_start(wo_sb, wo[:, :].rearrange("(kt p) e -> p kt e", p=hd))
            bqk_sb = sb.tile([P, hl], F32)
            nc.sync.dma_start(bqk_sb, bqk[:, :])
            bv_sb = sb.tile([1, hl * vw], F32)
            nc.sync.dma_start(bv_sb, bv[:, :])

            # ---- phase Q: projections ----
            q_sb = sb.tile([P, hp, s], BF16)  # pair p: head 2p at part 0-63, 2p+1 at 64-127
            k_sb = sb.tile([P, hp, s], BF16)
            for mt in range(2 * hp):          # [qp0, kp0, qp1, kp1]
                dst = q_sb if mt % 2 == 0 else k_sb
                for c in range(nch):
                    pq = ps.tile([P, ch], F32, tag="acc")
                    for kt in range(nkt):
                        nc.tensor.matmul(
                            pq,
                            wqk_sb[:, kt, mt * P:(mt + 1) * P],
                            xT_t(kt)[:, c * ch:(c + 1) * ch],
                            start=(kt == 0), stop=(kt == nkt - 1),
                        )
                    # copy + per-partition bias add (bias is zero in practice)
                    nc.vector.tensor_scalar_add(
                        dst[:, mt // 2, c * ch:(c + 1) * ch], pq,
                        bqk_sb[:, mt:mt + 1])
            v_sb = sb.tile([P, nst, hl * vw], BF16)
            for st in range(nst):
                pvq = ps.tile([P, ch], F32, tag="acc")
                for kt in range(nkt):
                    nc.tensor.matmul(
                        pvq[:, 0:hl * hd],
                        xT_t(kt)[:, st * P:(st + 1) * P],
                        wv_sb[:, kt, :],
                        start=(kt == 0), stop=(kt == nkt - 1),
                    )
                nc.vector.tensor_copy(
                    v_sb[:, st, :].rearrange("p (h e) -> p h e", e=vw)[:, :, 0:hd],
                    pvq[:, 0:hl * hd].rearrange("p (h e) -> p h e", e=hd),
                )
            nc.vector.memset(
                v_sb[:, :, :].rearrange("p st (h e) -> p st h e", e=vw)[:, :, :, hd],
                1.0)
            # v bias add (zero in practice): bv broadcast over partitions and st
            bv_bc = sb.tile([P, hl * vw], F32)
            nc.gpsimd.partition_broadcast(bv_bc, bv_sb)
            nc.vector.tensor_add(
                v_sb, v_sb,
                bv_bc[:, :].rearrange("p (o e) -> p o e", o=1)
                .broadcast_to([P, nst, hl * vw]))

            # ---- phase A ----
            vals_sb = sb.tile([hd, hl, s], BF16)
            o_sb_box = [None]

            def emit_outT_mt(c_, mt):
                # one d-tile of chunk c_'s output projection; inputs were
                # finalized a chunk ago so nothing stalls in-order engines
                if mt == 0:
                    o_sb_box[0] = sb2.tile([P, nmt, ch], F32, tag="attn",
                                           bufs=3, name="o_sb")
                o_sb = o_sb_box[0]
                po = ps.tile([P, ch], F32, tag="po", bufs=1, name="po")
                for kt in range(hl):
                    nc.tensor.matmul(
                        po,
                        wo_sb[:, kt, mt * P:(mt + 1) * P],
                        vals_sb[:, kt, c_ * ch:(c_ + 1) * ch],
                        start=(kt == 0), stop=(kt == hl - 1),
                    )
                nc.scalar.copy(o_sb[:, mt, :], po)
                if mt == nmt - 1:
                    nc.sync.dma_start(
                        outT[:, :].rearrange("(mt p) x -> p mt x", p=P)
                        [:, :, c_ * ch:(c_ + 1) * ch], o_sb)

            for c in range(nch):
                for pr in range(hp):
                    eh = [None, None]
                    pv0 = ps.tile([vw, ch], F32, tag="acc")
                    pv1 = ps.tile([vw, ch], F32, tag="acc2", bufs=1)
                    pv = (pv0, pv1)

                    def vals_mm(skt_):
                        cur_ = eh[skt_ // hst]
                        for hh in range(2):
                            h_ = 2 * pr + hh
                            nc.tensor.matmul(
                                pv[hh],
                                v_sb[:, skt_, h_ * vw:(h_ + 1) * vw],
                                cur_[:, skt_ % hst, hh, :],
                                start=(skt_ == 0), stop=(skt_ == nst - 1),
                            )

                    # vals matmuls run one sk tile behind the logits/exp so
                    # the in-order PE never waits on ACT; the previous chunk's
                    # output projection is spread through the middle.
                    for skt in range(nst):
                        if skt % hst == 0:
                            eh[skt // hst] = sb2.tile(
                                [P, hst, 2, ch], BF16, tag="exp", bufs=4,
                                name="eh")
                        cur = eh[skt // hst]
                        pl = ps.tile([P, 2, ch], F32, tag="l")
                        for hh in range(2):
                            b0 = hh * hd
                            nc.tensor.matmul(
                                pl[:, hh, :],
                                k_sb[b0:b0 + hd, pr, skt * P:(skt + 1) * P],
                                q_sb[b0:b0 + hd, pr, c * ch:(c + 1) * ch],
                                start=True, stop=True,
                            )
                        nc.scalar.activation(cur[:, skt % hst, :, :], pl, EXP,
                                             scale=1.0 / hd)
                        if skt >= 1:
                            vals_mm(skt - 1)
                        if pr == 0 and c > 0 and nst - 1 >= nmt:
                            st0 = max(1, min(4, nst - nmt))
                            if st0 <= skt < st0 + nmt:
                                emit_outT_mt(c - 1, skt - st0)
                    vals_mm(nst - 1)
                    if pr == 0 and c > 0 and nst - 1 < nmt:
                        for _m in range(nmt):
                            emit_outT_mt(c - 1, _m)

                    # Reciprocal chains for both heads, stages interleaved so
                    # one head's DMA/gpsimd hops hide under the other's DVE
                    # work. (HW partition_broadcast reads physical partition
                    # 0; the [1, ch] row is spread over 128 partitions so the
                    # reciprocal uses every DVE lane.)
                    sums64 = [None, None]
                    sums_sq = [None, None]
                    recip_sq = [None, None]
                    recip = [None, None]
                    recip_bc = [None, None]
                    for hh in range(2):
                        sums64[hh] = sb2.tile([P, ch], F32, tag="sums64",
                                              bufs=4, name=f"sums64_{hh}")
                        nc.scalar.copy(sums64[hh][hd:hd + 1, :],
                                       pv[hh][hd:hd + 1, :])
                        sums_sq[hh] = sb2.tile([P, cl], F32, tag="sums_sq",
                                               bufs=4, name=f"sums_sq_{hh}")
                        nc.gpsimd.dma_start(sums_sq[hh],
                                            sums64[hh][hd:hd + 1, :])
                    for hh in range(2):
                        recip_sq[hh] = sb2.tile([P, cl], F32, tag="recip_sq",
                                                bufs=4, name=f"recip_sq_{hh}")
                        nc.vector.reciprocal(recip_sq[hh], sums_sq[hh])
                        recip[hh] = sb2.tile([1, ch], F32, tag="recip",
                                             bufs=4, name=f"recip_{hh}")
                        nc.gpsimd.dma_start(recip[hh], recip_sq[hh])
                    for hh in range(2):
                        recip_bc[hh] = sb2.tile([P, ch], F32, tag="rbc",
                                                bufs=4, name=f"recip_bc_{hh}")
                        nc.gpsimd.partition_broadcast(recip_bc[hh], recip[hh])
                    for hh in range(2):
                        h = 2 * pr + hh
                        # vals row-normalize fused into the PSUM->SBUF copy
                        nc.vector.tensor_tensor(
                            vals_sb[:, h, c * ch:(c + 1) * ch],
                            pv[hh][0:hd, :], recip_bc[hh][0:hd, :],
                            op=mybir.AluOpType.mult)
                    # attn normalize + store; half-major so the older exp half
                    # frees its slot first
                    for half in range(2):
                        for hh in range(2):
                            h = 2 * pr + hh
                            attn_st = sb2.tile([P, hst, ch], F32, tag="attn",
                                               bufs=3)
                            nc.vector.tensor_tensor(
                                attn_st,
                                eh[half][:, :, hh, :],
                                recip_bc[hh][:, :]
                                .rearrange("p (o n) -> p o n", o=1)
                                .broadcast_to([P, hst, ch]),
                                op=mybir.AluOpType.mult)
                            nc.sync.dma_start(
                                attnT[h].rearrange("(t p) n -> p t n", p=P)
                                [:, half * hst:(half + 1) * hst,
                                 c * ch:(c + 1) * ch],
                                attn_st)

            for _mt in range(nmt):
                emit_outT_mt(nch - 1, _mt)

    nc.compile()
    return nc


def _get_nc():
    if "full" not in _NC_CACHE:
        _NC_CACHE["full"] = build_nc()
    return _NC_CACHE["full"]


def kernel(x, w_qkv, b_qkv, w_out, b_out):
    global LAST_RESULTS
    _install_ntff_hook()
    x = np.asarray(x, dtype=np.float32)
    w_qkv = np.asarray(w_qkv, dtype=np.float32)
    b_qkv = np.asarray(b_qkv, dtype=np.float32)
    w_out = np.asarray(w_out, dtype=np.float32)
    b_out = np.asarray(b_out, dtype=np.float32)

    # w_qkv rows are per-head interleaved: row h*192+j -> j<64: q, <128: k, <192: v
    wq = np.stack([w_qkv[g * 3 * HD + 0 * HD: g * 3 * HD + 1 * HD] for g in range(H)])
    wk = np.stack([w_qkv[g * 3 * HD + 1 * HD: g * 3 * HD + 2 * HD] for g in range(H)])
    wv_ = np.stack([w_qkv[g * 3 * HD + 2 * HD: g * 3 * HD + 3 * HD] for g in range(H)])
    bq = np.stack([b_qkv[g * 3 * HD + 0 * HD: g * 3 * HD + 1 * HD] for g in range(H)])
    bk = np.stack([b_qkv[g * 3 * HD + 1 * HD: g * 3 * HD + 2 * HD] for g in range(H)])
    bv_ = np.stack([b_qkv[g * 3 * HD + 2 * HD: g * 3 * HD + 3 * HD] for g in range(H)])

    in_maps = []
    for core in range(8):
        b = core // 4
        hg = core % 4
        gh = [hg * HL + i for i in range(HL)]     # global head ids
        xT = np.ascontiguousarray(x[b].T).astype(ml_dtypes.bfloat16)
        cols = []
        bqk_cols = []
        for p_ in range(HL // 2):
            h0, h1 = gh[2 * p_], gh[2 * p_ + 1]
            cols.append(np.concatenate([wq[h0], wq[h1]], axis=0))   # [128, D]
            bqk_cols.append(np.concatenate([bq[h0], bq[h1]]))
            cols.append(np.concatenate([wk[h0], wk[h1]], axis=0))
            bqk_cols.append(np.concatenate([bk[h0], bk[h1]]))
        wqk_arr = np.concatenate(cols, axis=0).T                     # [D, 512]
        bqk_arr = np.stack(bqk_cols, axis=1)                         # [128, 4]
        wv_arr = np.concatenate([wv_[g] for g in gh], axis=0).T      # [D, 256]
        bv_arr = np.zeros((1, HL * (HD + 1)), np.float32)
        for i, g in enumerate(gh):
            bv_arr[0, i * (HD + 1): i * (HD + 1) + HD] = bv_[g]
        wo_arr = np.ascontiguousarray(
            w_out[:, gh[0] * HD:(gh[-1] + 1) * HD].T)                # [256, D]
        in_maps.append({
            "xT": xT,
            "wqk": np.ascontiguousarray(wqk_arr).astype(ml_dtypes.bfloat16),
            "wv": np.ascontiguousarray(wv_arr).astype(ml_dtypes.bfloat16),
            "wo": wo_arr.astype(ml_dtypes.bfloat16),
            "bqk": np.ascontiguousarray(bqk_arr),
            "bv": bv_arr,
        })

    nc = _get_nc()
    res = run_bass_kernel_spmd(nc, in_maps, core_ids=list(range(8)))
    LAST_RESULTS = res

    attn = np.empty((B, H, S, S), np.float32)
    out = np.zeros((B, S, D), np.float32)
    for core in range(8):
        b = core // 4
        hg = core % 4
        r = res.results[core]
        for i in range(HL):
            attn[b, hg * HL + i] = r["attnT"][i].T
        out[b] += r["outT"].T
    out += b_out
    return out, attn
